# revision 13
# baseline (speedup 1.0000x reference)
"""Causal self-attention (B=2, T=2048, C=1024, NH=16, HS=64) on 8 TRN2 NeuronCores.

Sharding: core c -> batch b = c//4, head-group g = c%4 (4 heads per core).
Each core computes the qkv projection for its 768 W columns + causal attention
for its 4 heads; the host concatenates the per-core [T, 256] outputs.

v2 speed strategy (vs the 171us baseline):
  - x is transposed to xT on the HOST (input prep, like the fp16 cast), so the
    device does plain full-bandwidth DMA loads instead of serialized xbar
    transposes; W/x loads are chunked per-128-row block and split across the
    two HWDGE queues (SP + Activation) so the first projection matmul starts
    ~2us in instead of ~20us.
  - The qkv projection is software-interleaved WITH the attention loop: the
    attention steady state is ScalarE(exp)-bound (~1us/step vs ~0.64us of PE
    work), so projection matmuls are drained between attention steps via a
    credit scheduler to keep the PE saturated (and the HAM clock warm).
  - q, k produced transposed ([d, t]); scoresT[k, q] = kT.T @ qT so softmax's
    reduction dim is on partitions and PV contracts it directly. Head pairs
    pack the QK matmuls in one PE pass via tile_position row groups; a wide
    ScalarE Exp (fused 1/sqrt(HS)) covers the pair.
  - v natural [t, d] with a ones column -> PV emits [65, q]: rows 0:64 =
    head-output^T, row 64 = softmax sums. Causal masking: suffix-sliced
    matmuls + fp16 triangular-mask multiply on diagonal blocks; no row-max
    subtraction (scores bounded, fp16 exp is safe).
  - Finalize without any transpose on device: the [65, 512] PSUM block is
    normalized in place (reciprocal of the sums row, DMA-replicated across
    partitions, one DVE multiply) and stored transposed as outT [256, 2048]
    fp16; the host transposes while assembling the full output (pure data
    rearrangement, the inverse of the input prep).
All matmuls fp16 operands with fp32 PSUM accumulation.
"""
import sys

sys.path.insert(0, "/opt/trn_rl_repo")

from collections import deque

import numpy as np

import concourse.bass as bass
import concourse.tile as tile
from concourse import bacc, mybir
from concourse import bass_utils
from concourse.bass import ds, ts

B, T, C, NH, HS = 2, 2048, 1024, 16, 64
NCORES = 8
HPC = NH // 4  # heads per core = 4
GCOLS = HPC * HS  # 256 W columns per section per core
F32 = mybir.dt.float32
AF = mybir.ActivationFunctionType
ALU = mybir.AluOpType
DT_ATT = mybir.dt.float16

P = 128
KS = C // P  # 8 contraction subtiles
NTT = T // P  # 16 t-tiles
QCS = (0, 512, 1024, 1536)


def _emit(tc, nc, xT_d, w, bvec, out_d):
    import contextlib
    _stack = contextlib.ExitStack()
    singles = _stack.enter_context(tc.tile_pool(name="singles", bufs=1))

    # tri[k, m] = 1 if m >= k else 0  (keep upper-incl-diag of the 128x128
    # diagonal block in scoresT layout)
    tri = singles.tile([P, P], DT_ATT)
    nc.vector.memset(tri[:], 1.0)
    nc.gpsimd.affine_select(
        out=tri[:], in_=tri[:], compare_op=ALU.is_ge, fill=0.0,
        base=0, pattern=[[1, P]], channel_multiplier=-1,
    )

    # per-partition bias tiles for the transposed q/k layouts
    bq = [singles.tile([P, 1], F32, tag=f"bq{p}", name=f"bq{p}") for p in range(2)]
    bk = [singles.tile([P, 1], F32, tag=f"bk{p}", name=f"bk{p}") for p in range(2)]
    for p in range(2):
        nc.sync.dma_start(bq[p][:], bvec[ds(p * P, P)].rearrange("(p o) -> p o", o=1))
        nc.sync.dma_start(bk[p][:], bvec[ds(GCOLS + p * P, P)].rearrange("(p o) -> p o", o=1))
    bv = singles.tile([P, HPC, HS], F32)
    _bv_src = bvec[ds(2 * GCOLS, GCOLS)].rearrange("(h d) -> h d", h=HPC)
    nc.sync.dma_start(bv[:], bass.AP(tensor=_bv_src.tensor, offset=_bv_src.offset,
                                     ap=[[0, P], *_bv_src.ap]))

    # W and xT staged in SBUF, chunked per-ko so compute starts after the
    # first chunk. Chunks alternate between the two HWDGE queues (SP, ACT);
    # the ACT queue is only used up front, before the exps begin.
    wsb = singles.tile([P, KS, 3 * GCOLS], DT_ATT)
    xsb = singles.tile([P, KS, T], DT_ATT)
    qeng = [nc.sync, nc.scalar]
    for ko in range(KS):
        qeng[ko % 2].dma_start(wsb[:, ko, :], w[ds(ko * P, P), :])
    for tg in range(NTT // 4):
        for ko in range(KS):
            e = qeng[ko % 2] if tg == 0 else nc.sync
            e.dma_start(xsb[:, ko, ts(tg, 512)], xT_d[ds(ko * P, P), ts(tg, 512)])

    qT = singles.tile([P, 2, T], DT_ATT)
    kT = singles.tile([P, 2, T], DT_ATT)
    vA = singles.tile([P, NTT, HPC, HS + 1], DT_ATT)
    ones64 = singles.tile([P, NTT * HPC], F32)
    nc.vector.memset(ones64[:], 1.0)
    nc.vector.tensor_copy(
        vA[:, :, :, HS:HS + 1].rearrange("p a b o -> p (a b o)"), ones64[:]
    )

    with (
        tc.tile_pool(name="ps_proj", bufs=2, space="PSUM") as ps_proj,
        tc.tile_pool(name="ps_sc", bufs=2, space="PSUM") as ps_sc,
        tc.tile_pool(name="ps_pv", bufs=1, space="PSUM") as ps_pv,
        tc.tile_pool(name="wei", bufs=12) as weip,
        tc.tile_pool(name="fin", bufs=2) as finp,
    ):
        # ---- projection units, drained between attention steps -----------
        # Each item: (kind, idx, cost, emit_fn). kind 'q'/'k' indexed by tg,
        # kind 'v' indexed by tt. Attention steps force-drain exactly what
        # they depend on; leftover capacity drains via per-step credit.
        proj_stream = []

        def make_qk_unit(tg, sec, dstT, btile, pair, kind):
            state = {}

            def mk(k):
                def f():
                    if k == 0:
                        state["pq"] = ps_proj.tile(
                            [P, 512], F32, tag="pq", name=f"pq{tg}_{sec}_{pair}")
                    nc.tensor.matmul(
                        state["pq"][:],
                        wsb[:, k, ds(sec + pair * P, P)],
                        xsb[:, k, ts(tg, 512)],
                        start=(k == 0), stop=(k == KS - 1),
                    )
                    if k == KS - 1:
                        nc.vector.tensor_scalar_add(
                            dstT[:, pair, ts(tg, 512)], state["pq"][:], btile[:]
                        )
                return f
            return [(kind, tg, 1.0, mk(k)) for k in range(KS)]

        def make_v_unit(tg, i):
            tt = tg * 4 + i
            state = {}

            def mk(k):
                def f():
                    if k == 0:
                        state["pv"] = ps_proj.tile(
                            [P, 512], F32, tag="pq", name=f"pvp{tt}")
                    nc.tensor.matmul(
                        state["pv"][:, 0:GCOLS],
                        xsb[:, k, ts(tt, P)],
                        wsb[:, k, ds(2 * GCOLS, GCOLS)],
                        start=(k == 0), stop=(k == KS - 1),
                    )
                    if k == KS - 1:
                        nc.vector.tensor_tensor(
                            vA[:, tt, :, 0:HS],
                            state["pv"][:, 0:GCOLS].rearrange("p (h d) -> p h d", h=HPC),
                            bv[:],
                            ALU.add,
                        )
                return f
            return [("v", tt, 0.5, mk(k)) for k in range(KS)]

        for tg in range(NTT // 4):
            for pair in range(2):
                proj_stream += make_qk_unit(tg, GCOLS, kT, bk[pair], pair, "k")
            for pair in range(2):
                proj_stream += make_qk_unit(tg, 0, qT, bq[pair], pair, "q")
            for i in range(4):
                proj_stream += make_v_unit(tg, i)

        cursor = [0]
        done = {"q": -1, "k": -1, "v": -1}

        def _emit_next():
            kind, idx, cost, fn = proj_stream[cursor[0]]
            fn()
            cursor[0] += 1
            # a unit of this kind is fully emitted when the next item differs
            if cursor[0] >= len(proj_stream):
                done["q"] = done["k"] = 999
                done["v"] = 999
            else:
                nk, ni, _, _ = proj_stream[cursor[0]]
                if (nk, ni) != (kind, idx):
                    done[kind] = max(done[kind], idx)
            return cost

        def force(kind, idx):
            while done[kind] < idx and cursor[0] < len(proj_stream):
                _emit_next()

        def drain(credit):
            while credit > 0 and cursor[0] < len(proj_stream):
                credit -= _emit_next()

        # ---- attention ---------------------------------------------------
        jmaxes = {qc: min(NTT - 1, qc // P + 3) for qc in QCS}
        pvh_tiles = {}
        wei_tiles = {}

        def step_qk(pair, qc, j):
            diag = (j * P) // 512 * 512 == qc
            o = j * P - qc if diag else 0
            s = ps_sc.tile([P, 1024], F32, tag="scps", name=f"sc{pair}_{qc}_{j}")
            wei = weip.tile([P, 1024], DT_ATT, tag="wei", name=f"wei{pair}_{qc}_{j}")
            for hh in range(2):
                nc.tensor.matmul(
                    s[:, hh * 512 + o:hh * 512 + 512],
                    kT[ds(hh * HS, HS), pair, ts(j, P)],
                    qT[ds(hh * HS, HS), pair, ds(qc + o, 512 - o)],
                    start=True, stop=True,
                    tile_position=(hh * HS, 0),
                )
            if o == 0:
                nc.scalar.activation(
                    wei[:], s[:], AF.Exp, scale=float(HS) ** -0.5
                )
            else:
                for hh in range(2):
                    nc.scalar.activation(
                        wei[:, hh * 512 + o:hh * 512 + 512],
                        s[:, hh * 512 + o:hh * 512 + 512],
                        AF.Exp, scale=float(HS) ** -0.5,
                    )
            if diag:
                for hh in range(2):
                    nc.vector.tensor_tensor(
                        wei[:, ds(hh * 512 + o, P)],
                        wei[:, ds(hh * 512 + o, P)], tri[:], ALU.mult
                    )
            wei_tiles[(pair, qc, j)] = (wei, o)

        def emit_fin(pair, qc):
            for hh in range(2):
                h = pair * 2 + hh
                pvs = pvh_tiles.pop((pair, qc, hh))
                # reciprocal of the softmax-sum row, in place on partition 64
                rec = finp.tile([HS + 1, 512], F32, tag="rec",
                                name=f"rec{pair}_{qc}_{hh}")
                nc.vector.reciprocal(rec[HS:HS + 1, :], pvs[HS:HS + 1, :])
                # replicate the [1, 512] row across 64 partitions via DMA
                recB = finp.tile([HS, 512], F32, tag="recB",
                                 name=f"recB{pair}_{qc}_{hh}")
                _src = rec[HS:HS + 1, :]
                nc.sync.dma_start(
                    recB[:],
                    bass.AP(tensor=_src.tensor, offset=_src.offset,
                            ap=[_src.ap[0], [0, HS], _src.ap[-1]]),
                )
                fo = finp.tile([HS, 512], DT_ATT, tag="fo",
                               name=f"fo{pair}_{qc}_{hh}")
                nc.vector.tensor_tensor(fo[:], pvs[0:HS, :], recB[:], ALU.mult)
                nc.gpsimd.dma_start(out_d[ds(h * HS, HS), ds(qc, 512)], fo[:])

        def step_pv(pair, qc, j):
            jmax = jmaxes[qc]
            if j == 0:
                for hh in range(2):
                    pvh_tiles[(pair, qc, hh)] = ps_pv.tile(
                        [HS + 1, 512], F32, tag=f"pv{hh}",
                        name=f"pvps{pair}_{qc}_{hh}")
            wei, o = wei_tiles.pop((pair, qc, j))
            for hh in range(2):
                h = pair * 2 + hh
                nc.tensor.matmul(
                    pvh_tiles[(pair, qc, hh)][:, o:512],
                    vA[:, j, h, :],
                    wei[:, hh * 512 + o:hh * 512 + 512],
                    start=(j == 0), stop=(j == jmax),
                )
            if j == jmax:
                emit_fin(pair, qc)

        LAG = 4
        pending = deque()
        for qc in QCS:
            for pair in range(2):
                for j in range(jmaxes[qc] + 1):
                    force("q", qc // 512)
                    force("k", j // 4)
                    step_qk(pair, qc, j)
                    pending.append((pair, qc, j))
                    if len(pending) > LAG:
                        pj = pending.popleft()
                        force("v", pj[2])
                        step_pv(*pj)
                    drain(2.0)
        while pending:
            pj = pending.popleft()
            force("v", pj[2])
            step_pv(*pj)
        drain(1e9)

    _stack.close()


_CACHED_NC = None


def _build():
    global _CACHED_NC
    if _CACHED_NC is not None:
        return _CACHED_NC
    nc = bacc.Bacc("TRN2", target_bir_lowering=False, debug=False,
                   num_devices=NCORES)
    xT_d = nc.dram_tensor("xt", [C, T], DT_ATT, kind="ExternalInput").ap()
    w = nc.dram_tensor("w", [C, 3 * GCOLS], DT_ATT, kind="ExternalInput").ap()
    bvec = nc.dram_tensor("b", [3 * GCOLS], F32, kind="ExternalInput").ap()
    out_d = nc.dram_tensor("out", [GCOLS, T], DT_ATT, kind="ExternalOutput").ap()
    with tile.TileContext(nc) as tc:
        _emit(tc, nc, xT_d, w, bvec, out_d)
    nc.compile()
    _CACHED_NC = nc
    return nc


def _in_maps(x, W_attn, b_attn):
    x = np.asarray(x, dtype=np.float32)
    W = np.asarray(W_attn, dtype=np.float32)
    bias = np.asarray(b_attn, dtype=np.float32)
    maps = []
    for c in range(NCORES):
        b_idx, g = c // 4, c % 4
        cols = slice(g * GCOLS, (g + 1) * GCOLS)
        wc = np.concatenate(
            [W[:, cols], W[:, C:][:, cols], W[:, 2 * C:][:, cols]], axis=1
        )
        bc = np.concatenate(
            [bias[cols], bias[C:][cols], bias[2 * C:][cols]], axis=0
        )
        maps.append({
            "xt": np.ascontiguousarray(x[b_idx].T).astype(np.float16),
            "w": np.ascontiguousarray(wc).astype(np.float16),
            "b": np.ascontiguousarray(bc),
        })
    return maps


def run(x, W_attn, b_attn, trace=False):
    nc = _build()
    maps = _in_maps(x, W_attn, b_attn)
    res = bass_utils.run_bass_kernel_spmd(
        nc, maps, list(range(NCORES)), trace=trace,
        trace_cores=[0] if trace else None,
    )
    out = np.empty((B, T, C), dtype=np.float32)
    for c in range(NCORES):
        b_idx, g = c // 4, c % 4
        out[b_idx, :, g * GCOLS:(g + 1) * GCOLS] = res.results[c]["out"].T.astype(np.float32)
    return out, res


def kernel(x, W_attn, b_attn):
    out, _ = run(x, W_attn, b_attn, trace=False)
    return out


# revision 19
# speedup vs baseline: 1.2299x; 1.2299x over previous
"""Causal self-attention (B=2, T=2048, C=1024, NH=16, HS=64) on 8 TRN2 NeuronCores.

Sharding: core c -> batch b = c//4, head-group g = c%4 (4 heads per core).
Each core computes the qkv projection for its 768 W columns + causal attention
for its 4 heads; the host concatenates the per-core [T, 256] outputs.

v2 speed strategy (vs the 171us baseline):
  - x is transposed to xT on the HOST (input prep, like the fp16 cast), so the
    device does plain full-bandwidth DMA loads instead of serialized xbar
    transposes; W/x loads are chunked per-128-row block and split across the
    two HWDGE queues (SP + Activation) so the first projection matmul starts
    ~2us in instead of ~20us.
  - The qkv projection is software-interleaved WITH the attention loop: the
    attention steady state is ScalarE(exp)-bound (~1us/step vs ~0.64us of PE
    work), so projection matmuls are drained between attention steps via a
    credit scheduler to keep the PE saturated (and the HAM clock warm).
  - q, k produced transposed ([d, t]); scoresT[k, q] = kT.T @ qT so softmax's
    reduction dim is on partitions and PV contracts it directly. Head pairs
    pack the QK matmuls in one PE pass via tile_position row groups; a wide
    ScalarE Exp (fused 1/sqrt(HS)) covers the pair.
  - v natural [t, d] with a ones column -> PV emits [65, q]: rows 0:64 =
    head-output^T, row 64 = softmax sums. Causal masking: suffix-sliced
    matmuls + fp16 triangular-mask multiply on diagonal blocks; no row-max
    subtraction (scores bounded, fp16 exp is safe).
  - Finalize without any transpose on device: the [65, 512] PSUM block is
    normalized in place (reciprocal of the sums row, DMA-replicated across
    partitions, one DVE multiply) and stored transposed as outT [256, 2048]
    fp16; the host transposes while assembling the full output (pure data
    rearrangement, the inverse of the input prep).
All matmuls fp16 operands with fp32 PSUM accumulation.
"""
import sys

sys.path.insert(0, "/opt/trn_rl_repo")

from collections import deque

import numpy as np

import concourse.bass as bass
import concourse.tile as tile
from concourse import bacc, mybir
from concourse import bass_utils
from concourse.bass import ds, ts

B, T, C, NH, HS = 2, 2048, 1024, 16, 64
NCORES = 8
HPC = NH // 4  # heads per core = 4
GCOLS = HPC * HS  # 256 W columns per section per core
F32 = mybir.dt.float32
AF = mybir.ActivationFunctionType
ALU = mybir.AluOpType
DT_ATT = mybir.dt.float16

P = 128
KS = C // P  # 8 contraction subtiles
NTT = T // P  # 16 t-tiles
QCS = (0, 512, 1024, 1536)


def _emit(tc, nc, xT_d, w, bvec, out_d):
    import contextlib
    _stack = contextlib.ExitStack()
    singles = _stack.enter_context(tc.tile_pool(name="singles", bufs=1))

    # tri[k, m] = 1 if m >= k else 0  (keep upper-incl-diag of the 128x128
    # diagonal block in scoresT layout)
    tri = singles.tile([P, P], DT_ATT)
    nc.vector.memset(tri[:], 1.0)
    nc.gpsimd.affine_select(
        out=tri[:], in_=tri[:], compare_op=ALU.is_ge, fill=0.0,
        base=0, pattern=[[1, P]], channel_multiplier=-1,
    )

    # per-partition bias tiles for the transposed q/k layouts
    bq = [singles.tile([P, 1], F32, tag=f"bq{p}", name=f"bq{p}") for p in range(2)]
    bk = [singles.tile([P, 1], F32, tag=f"bk{p}", name=f"bk{p}") for p in range(2)]
    for p in range(2):
        nc.sync.dma_start(bq[p][:], bvec[ds(p * P, P)].rearrange("(p o) -> p o", o=1))
        nc.sync.dma_start(bk[p][:], bvec[ds(GCOLS + p * P, P)].rearrange("(p o) -> p o", o=1))
    bv = singles.tile([P, HPC, HS], F32)
    _bv_src = bvec[ds(2 * GCOLS, GCOLS)].rearrange("(h d) -> h d", h=HPC)
    nc.sync.dma_start(bv[:], bass.AP(tensor=_bv_src.tensor, offset=_bv_src.offset,
                                     ap=[[0, P], *_bv_src.ap]))

    # W and xT staged in SBUF, chunked per-ko so compute starts after the
    # first chunk. Chunks alternate between the two HWDGE queues (SP, ACT);
    # the ACT queue is only used up front, before the exps begin.
    wsb = singles.tile([P, KS, 3 * GCOLS], DT_ATT)
    xsb = singles.tile([P, KS, T], DT_ATT)
    qeng = [nc.sync, nc.scalar]
    for ko in range(KS):
        qeng[ko % 2].dma_start(wsb[:, ko, :], w[ds(ko * P, P), :])
    for tg in range(NTT // 4):
        for ko in range(KS):
            e = qeng[ko % 2] if tg < 2 else nc.sync
            e.dma_start(xsb[:, ko, ts(tg, 512)], xT_d[ds(ko * P, P), ts(tg, 512)])

    qT = singles.tile([P, 2, T], DT_ATT)
    kT = singles.tile([P, 2, T], DT_ATT)
    vA = singles.tile([P, NTT, HPC, HS + 1], DT_ATT)
    ones64 = singles.tile([P, NTT * HPC], F32)
    nc.vector.memset(ones64[:], 1.0)
    nc.vector.tensor_copy(
        vA[:, :, :, HS:HS + 1].rearrange("p a b o -> p (a b o)"), ones64[:]
    )

    with (
        tc.tile_pool(name="ps_proj", bufs=2, space="PSUM") as ps_proj,
        tc.tile_pool(name="ps_sc", bufs=2, space="PSUM") as ps_sc,
        tc.tile_pool(name="ps_pv", bufs=1, space="PSUM") as ps_pv,
        tc.tile_pool(name="wei", bufs=12) as weip,
        tc.tile_pool(name="fin", bufs=2) as finp,
    ):
        # ---- projection units, drained between attention steps -----------
        # Each item: (kind, idx, cost, emit_fn). kind 'q'/'k' indexed by tg,
        # kind 'v' indexed by tt. Attention steps force-drain exactly what
        # they depend on; leftover capacity drains via per-step credit.
        proj_stream = []

        def make_qk_unit(tg, sec, dstT, btile, pair, kind):
            state = {}

            def mk(k):
                def f():
                    if k == 0:
                        state["pq"] = ps_proj.tile(
                            [P, 512], F32, tag="pq", name=f"pq{tg}_{sec}_{pair}")
                    nc.tensor.matmul(
                        state["pq"][:],
                        wsb[:, k, ds(sec + pair * P, P)],
                        xsb[:, k, ts(tg, 512)],
                        start=(k == 0), stop=(k == KS - 1),
                    )
                    if k == KS - 1:
                        nc.vector.tensor_scalar_add(
                            dstT[:, pair, ts(tg, 512)], state["pq"][:], btile[:]
                        )
                return f
            return [(kind, tg, 1.0, mk(k)) for k in range(KS)]

        def make_v_unit(tg, i):
            tt = tg * 4 + i
            state = {}

            def mk(k):
                def f():
                    if k == 0:
                        state["pv"] = ps_proj.tile(
                            [P, 512], F32, tag="pq", name=f"pvp{tt}")
                    nc.tensor.matmul(
                        state["pv"][:, 0:GCOLS],
                        xsb[:, k, ts(tt, P)],
                        wsb[:, k, ds(2 * GCOLS, GCOLS)],
                        start=(k == 0), stop=(k == KS - 1),
                    )
                    if k == KS - 1:
                        nc.vector.tensor_tensor(
                            vA[:, tt, :, 0:HS],
                            state["pv"][:, 0:GCOLS].rearrange("p (h d) -> p h d", h=HPC),
                            bv[:],
                            ALU.add,
                        )
                return f
            return [("v", tt, 0.5, mk(k)) for k in range(KS)]

        for tg in range(NTT // 4):
            for pair in range(2):
                proj_stream += make_qk_unit(tg, GCOLS, kT, bk[pair], pair, "k")
            for pair in range(2):
                proj_stream += make_qk_unit(tg, 0, qT, bq[pair], pair, "q")
            for i in range(4):
                proj_stream += make_v_unit(tg, i)

        cursor = [0]
        done = {"q": -1, "k": -1, "v": -1}

        def _emit_next():
            kind, idx, cost, fn = proj_stream[cursor[0]]
            fn()
            cursor[0] += 1
            # a unit of this kind is fully emitted when the next item differs
            if cursor[0] >= len(proj_stream):
                done["q"] = done["k"] = 999
                done["v"] = 999
            else:
                nk, ni, _, _ = proj_stream[cursor[0]]
                if (nk, ni) != (kind, idx):
                    done[kind] = max(done[kind], idx)
            return cost

        def force(kind, idx):
            while done[kind] < idx and cursor[0] < len(proj_stream):
                _emit_next()

        def drain(credit):
            while credit > 0 and cursor[0] < len(proj_stream):
                credit -= _emit_next()

        # ---- attention ---------------------------------------------------
        jmaxes = {qc: min(NTT - 1, qc // P + 3) for qc in QCS}
        pvh_tiles = {}
        wei_tiles = {}

        def step_qk(pair, qc, j):
            diag = (j * P) // 512 * 512 == qc
            o = j * P - qc if diag else 0
            s = ps_sc.tile([P, 1024], F32, tag="scps", name=f"sc{pair}_{qc}_{j}")
            wei = weip.tile([P, 1024], DT_ATT, tag="wei", name=f"wei{pair}_{qc}_{j}")
            for hh in range(2):
                nc.tensor.matmul(
                    s[:, hh * 512 + o:hh * 512 + 512],
                    kT[ds(hh * HS, HS), pair, ts(j, P)],
                    qT[ds(hh * HS, HS), pair, ds(qc + o, 512 - o)],
                    start=True, stop=True,
                    tile_position=(hh * HS, 0),
                )
            if o == 0:
                nc.scalar.activation(
                    wei[:], s[:], AF.Exp, scale=float(HS) ** -0.5
                )
            else:
                for hh in range(2):
                    nc.scalar.activation(
                        wei[:, hh * 512 + o:hh * 512 + 512],
                        s[:, hh * 512 + o:hh * 512 + 512],
                        AF.Exp, scale=float(HS) ** -0.5,
                    )
            if diag:
                for hh in range(2):
                    nc.vector.tensor_tensor(
                        wei[:, ds(hh * 512 + o, P)],
                        wei[:, ds(hh * 512 + o, P)], tri[:], ALU.mult
                    )
            wei_tiles[(pair, qc, j)] = (wei, o)

        def emit_fin(pair, qc):
            for hh in range(2):
                h = pair * 2 + hh
                pvs = pvh_tiles.pop((pair, qc, hh))
                # cast to fp16 SBUF (incl. sums row) — releases the PV PSUM bank
                ot = finp.tile([HS + 1, 512], DT_ATT, tag="ot",
                               name=f"ot{pair}_{qc}_{hh}")
                nc.vector.tensor_copy(ot[:], pvs[:])
                # reciprocal of the sums row: spread [1,512] -> [128,4] via DMA
                # (InstReciprocal cost scales with free size), recip, scatter
                # back to one row, then replicate across the 64 d-partitions.
                den4 = finp.tile([P, 4], DT_ATT, tag="den4",
                                 name=f"den4_{pair}_{qc}_{hh}")
                _dsrc = ot[HS:HS + 1, :]
                nc.sync.dma_start(
                    den4[:],
                    bass.AP(tensor=_dsrc.tensor, offset=_dsrc.offset,
                            ap=[_dsrc.ap[0], [4, P], [1, 4]]),
                )
                rec4 = finp.tile([P, 4], F32, tag="rec4",
                                 name=f"rec4_{pair}_{qc}_{hh}")
                nc.vector.reciprocal(rec4[:], den4[:])
                rec = finp.tile([1, 512], F32, tag="rec",
                                name=f"rec{pair}_{qc}_{hh}")
                _rdst = rec[0:1, :]
                nc.sync.dma_start(
                    bass.AP(tensor=_rdst.tensor, offset=_rdst.offset,
                            ap=[_rdst.ap[0], [4, P], [1, 4]]),
                    rec4[:],
                )
                recB = finp.tile([HS, 512], F32, tag="recB",
                                 name=f"recB{pair}_{qc}_{hh}")
                _src = rec[0:1, :]
                nc.sync.dma_start(
                    recB[:],
                    bass.AP(tensor=_src.tensor, offset=_src.offset,
                            ap=[_src.ap[0], [0, HS], _src.ap[-1]]),
                )
                fo = finp.tile([HS, 512], DT_ATT, tag="fo",
                               name=f"fo{pair}_{qc}_{hh}")
                nc.vector.tensor_tensor(fo[:], ot[0:HS, :], recB[:], ALU.mult)
                nc.gpsimd.dma_start(out_d[ds(h * HS, HS), ds(qc, 512)], fo[:])

        def step_pv(pair, qc, j):
            jmax = jmaxes[qc]
            if j == 0:
                for hh in range(2):
                    pvh_tiles[(pair, qc, hh)] = ps_pv.tile(
                        [HS + 1, 512], F32, tag=f"pv{hh}",
                        name=f"pvps{pair}_{qc}_{hh}")
            wei, o = wei_tiles.pop((pair, qc, j))
            for hh in range(2):
                h = pair * 2 + hh
                nc.tensor.matmul(
                    pvh_tiles[(pair, qc, hh)][:, o:512],
                    vA[:, j, h, :],
                    wei[:, hh * 512 + o:hh * 512 + 512],
                    start=(j == 0), stop=(j == jmax),
                )
            if j == jmax:
                emit_fin(pair, qc)

        LAG = 4
        pending = deque()
        for qc in QCS:
            for pair in range(2):
                for j in range(jmaxes[qc] + 1):
                    force("q", qc // 512)
                    force("k", j // 4)
                    step_qk(pair, qc, j)
                    pending.append((pair, qc, j))
                    if len(pending) > LAG:
                        pj = pending.popleft()
                        force("v", pj[2])
                        step_pv(*pj)
                    drain(2.0)
        while pending:
            pj = pending.popleft()
            force("v", pj[2])
            step_pv(*pj)
        drain(1e9)

    _stack.close()


_CACHED_NC = None


def _build():
    global _CACHED_NC
    if _CACHED_NC is not None:
        return _CACHED_NC
    nc = bacc.Bacc("TRN2", target_bir_lowering=False, debug=False,
                   num_devices=NCORES)
    xT_d = nc.dram_tensor("xt", [C, T], DT_ATT, kind="ExternalInput").ap()
    w = nc.dram_tensor("w", [C, 3 * GCOLS], DT_ATT, kind="ExternalInput").ap()
    bvec = nc.dram_tensor("b", [3 * GCOLS], F32, kind="ExternalInput").ap()
    out_d = nc.dram_tensor("out", [GCOLS, T], DT_ATT, kind="ExternalOutput").ap()
    with tile.TileContext(nc) as tc:
        _emit(tc, nc, xT_d, w, bvec, out_d)
    nc.compile()
    _CACHED_NC = nc
    return nc


def _in_maps(x, W_attn, b_attn):
    x = np.asarray(x, dtype=np.float32)
    W = np.asarray(W_attn, dtype=np.float32)
    bias = np.asarray(b_attn, dtype=np.float32)
    maps = []
    for c in range(NCORES):
        b_idx, g = c // 4, c % 4
        cols = slice(g * GCOLS, (g + 1) * GCOLS)
        wc = np.concatenate(
            [W[:, cols], W[:, C:][:, cols], W[:, 2 * C:][:, cols]], axis=1
        )
        bc = np.concatenate(
            [bias[cols], bias[C:][cols], bias[2 * C:][cols]], axis=0
        )
        maps.append({
            "xt": np.ascontiguousarray(x[b_idx].T).astype(np.float16),
            "w": np.ascontiguousarray(wc).astype(np.float16),
            "b": np.ascontiguousarray(bc),
        })
    return maps


def run(x, W_attn, b_attn, trace=False):
    nc = _build()
    maps = _in_maps(x, W_attn, b_attn)
    res = bass_utils.run_bass_kernel_spmd(
        nc, maps, list(range(NCORES)), trace=trace,
        trace_cores=[0] if trace else None,
    )
    out = np.empty((B, T, C), dtype=np.float32)
    for c in range(NCORES):
        b_idx, g = c // 4, c % 4
        out[b_idx, :, g * GCOLS:(g + 1) * GCOLS] = res.results[c]["out"].T.astype(np.float32)
    return out, res


def kernel(x, W_attn, b_attn):
    out, _ = run(x, W_attn, b_attn, trace=False)
    return out
